# revision 8
# baseline (speedup 1.0000x reference)
"""TRN2 Bass kernel for nn_DeepSeekPretrainedMoE (8-core tensor-parallel).

Algorithm (validated vs reference in numpy mirror, l2 rel ~1.4e-6):
  h1 = x@W_in + b_in; hn1 = rmsnorm(h1)*ln1  (ln1 folded into Wq/Wk/Wv)
  attention (4 heads/core, causal, softmax without max-subtraction),
  h2 = AllReduce(ctx@Wo_shard + h1/8); hn2 = rmsnorm(h2)*ln2 (folded)
  act = silu(hn2@Wg_shard) * (hn2@Wu_shard)      [FF column-sharded]
  rl16 = h2@[W_router|Sel8] + AllReduce(act@[W_down@W_router|W_down[:,:8]])
  top-2 of rl16[:8] -> gather rl16[8:] -> agg; out = agg*0.5*colsum(W_out)+b_out

Precision: all router-critical GEMMs in bf16 hi/lo 3-pass (err ~1e-5 rel).
Layout: feature-major activations [D, tokens]; 4 token-chunks of 512.
"""
import contextlib
import ctypes
import sys
import types

sys.path.insert(0, "/opt/trn_rl_repo")

import numpy as np
import ml_dtypes


def _install_ntff_hook():
    if "antenv.axon_hooks" in sys.modules:
        return
    hook = None
    try:
        lib = ctypes.CDLL("/opt/axon/libaxon_pjrt.so")
        if hasattr(lib, "axon_start_nrt_profile"):
            lib.axon_start_nrt_profile.argtypes = [
                ctypes.POINTER(ctypes.c_int64), ctypes.c_size_t]
            lib.axon_start_nrt_profile.restype = ctypes.c_int64
            lib.axon_stop_nrt_profile.argtypes = [ctypes.c_char_p]
            lib.axon_stop_nrt_profile.restype = ctypes.c_int64

            @contextlib.contextmanager
            def hook(output_dir, device_ids):
                import jax
                jax.devices()
                if device_ids:
                    ids = (ctypes.c_int64 * len(device_ids))(*device_ids)
                    rc = lib.axon_start_nrt_profile(ids, len(device_ids))
                else:
                    rc = lib.axon_start_nrt_profile(None, 0)
                if rc != 0:
                    raise RuntimeError(f"axon_start_nrt_profile rc={rc}")
                try:
                    yield
                finally:
                    n = lib.axon_stop_nrt_profile(str(output_dir).encode())
                    if n < 0:
                        raise RuntimeError(f"axon_stop_nrt_profile rc={n}")
    except OSError:
        pass
    mod = types.ModuleType("antenv.axon_hooks")
    mod.get_axon_ntff_profile_hook = lambda: hook

    def _set(h):
        mod.get_axon_ntff_profile_hook = lambda: h
    mod.set_axon_ntff_profile_hook = _set
    import antenv
    antenv.axon_hooks = mod
    sys.modules["antenv.axon_hooks"] = mod


_install_ntff_hook()

import concourse.bacc as bacc            # noqa: E402
import concourse.mybir as mybir          # noqa: E402
import concourse.tile as tile            # noqa: E402
from concourse.bass_utils import run_bass_kernel_spmd  # noqa: E402
from concourse.alu_op_type import AluOpType as OP      # noqa: E402
import bass_rust                          # noqa: E402

AF = bass_rust.ActivationFunctionType
AX = mybir.AxisListType
dt = mybir.dt
F32, BF16 = dt.float32, dt.bfloat16

B, S, DIN, D, H, DH, FF, E = 2, 1024, 512, 4096, 32, 128, 11008, 8
NCORE, HPC = 8, 4
FFP, FFS = 11264, 1408
NT = B * S
CH = 4
EPS = 1e-6
BF = ml_dtypes.bfloat16
P3 = ((0, 0), (1, 0), (0, 1))   # (w_half, x_half) 3-pass schedule

LAST_RESULT = None


def _split(a):
    hi = a.astype(BF)
    lo = (a.astype(np.float32) - hi.astype(np.float32)).astype(BF)
    return np.stack([hi, lo])


def _build():
    nc = bacc.Bacc("TRN2", target_bir_lowering=False)
    di = {}

    def inp(name, shape, d=BF16):
        di[name] = nc.dram_tensor(name, shape, d, kind="ExternalInput")

    inp("xT16", [2, 4, 128, NT])
    inp("Win16", [2, 32, 128, 4, 128])
    inp("Wq16", [2, 4, 128, 32, 128])
    inp("Wk16", [2, 4, 128, 32, 128])
    inp("Wv16", [2, 32, 128, 512])
    inp("Wo16", [2, 32, 128, 4, 128])
    inp("Wg16", [2, 11, 128, 32, 128])
    inp("Wu16", [2, 11, 128, 32, 128])
    inp("Wds16", [2, 128, 11, 16])
    inp("Wrs16", [2, 128, 32, 16])
    inp("masks16", [128, 4, 512])
    inp("bin_t", [128, 32], F32)
    inp("br16", [16, 1], F32)
    inp("wsumb", [128, 512], F32)
    inp("boutb", [128, 512], F32)
    inp("iota8b", [128, 8], F32)
    inp("ident", [128, 128], F32)
    out_d = nc.dram_tensor("out", [NT, 512], F32, kind="ExternalOutput")

    with contextlib.ExitStack() as _st:
        tc = _st.enter_context(tile.TileContext(nc))
        ec = _st.enter_context
        pp = ec(tc.tile_pool(name="persist", bufs=1))
        wst = ec(tc.tile_pool(name="wst", bufs=6))
        xp = ec(tc.tile_pool(name="xp", bufs=8))
        wgu = ec(tc.tile_pool(name="wgu", bufs=6))
        evp = ec(tc.tile_pool(name="ev", bufs=4))
        sqp = ec(tc.tile_pool(name="sqp", bufs=2))
        ppl = ec(tc.tile_pool(name="ppool", bufs=2))
        pbp = ec(tc.tile_pool(name="pb", bufs=4))
        sml = ec(tc.tile_pool(name="sml", bufs=3))
        rlp16 = ec(tc.tile_pool(name="rl16p", bufs=2))
        bcp = ec(tc.tile_pool(name="bc", bufs=2))
        fin = ec(tc.tile_pool(name="fin", bufs=10))
        otp = ec(tc.tile_pool(name="ot", bufs=2))
        h2l = ec(tc.tile_pool(name="h2l", bufs=2))
        ps_acc = ec(tc.tile_pool(name="ps_acc", bufs=4, space="PSUM"))
        ps_ctx = ec(tc.tile_pool(name="ps_ctx", bufs=1, space="PSUM"))
        ps_den = ec(tc.tile_pool(name="ps_den", bufs=1, space="PSUM"))
        ps_var = ec(tc.tile_pool(name="ps_var", bufs=1, space="PSUM"))
        ps_rl = ec(tc.tile_pool(name="ps_rl", bufs=1, space="PSUM"))
        dr = ec(tc.tile_pool(name="dram", bufs=1, space="DRAM"))
        if True:
            H1hi = pp.tile([128, 32, 512], BF16, tag="H1hi")
            H1lo = pp.tile([128, 32, 512], BF16, tag="H1lo")
            HP = (H1hi, H1lo)
            Khi = pp.tile([128, 4, 1024], BF16, tag="Khi")
            Klo = pp.tile([128, 4, 1024], BF16, tag="Klo")
            Vhi = pp.tile([128, 8, 512], BF16, tag="Vhi")
            Vlo = pp.tile([128, 8, 512], BF16, tag="Vlo")
            Qhi = pp.tile([128, 4, 512], BF16, tag="Qhi")
            Qlo = pp.tile([128, 4, 512], BF16, tag="Qlo")
            QP = (Qhi, Qlo)
            CXhi = pp.tile([128, 4, 512], BF16, tag="CXhi")
            CXlo = pp.tile([128, 4, 512], BF16, tag="CXlo")
            CXP = (CXhi, CXlo)
            ones16 = pp.tile([128, 1], BF16, tag="ones16")
            nc.vector.memset(ones16[:], 1.0)
            c99 = pp.tile([128, 8], F32, tag="c99")
            nc.vector.memset(c99[:], 99.0)
            negb = pp.tile([128, 8], F32, tag="negb")
            nc.vector.memset(negb[:], -1e30)
            zero8 = pp.tile([128, 8], F32, tag="zero8")
            nc.vector.memset(zero8[:], 0.0)
            maskt = pp.tile([128, 4, 512], BF16, tag="maskt")
            nc.sync.dma_start(maskt[:], di["masks16"][:, :, :])
            bin_t = pp.tile([128, 32], F32, tag="bin_t")
            nc.sync.dma_start(bin_t[:], di["bin_t"][:, :])
            br16 = pp.tile([16, 1], F32, tag="br16")
            nc.sync.dma_start(br16[:], di["br16"][:, :])
            wsumb = pp.tile([128, 512], F32, tag="wsumb")
            nc.sync.dma_start(wsumb[:], di["wsumb"][:, :])
            boutb = pp.tile([128, 512], F32, tag="boutb")
            nc.sync.dma_start(boutb[:], di["boutb"][:, :])
            iota8b = pp.tile([128, 8], F32, tag="iota8b")
            nc.sync.dma_start(iota8b[:], di["iota8b"][:, :])
            ident = pp.tile([128, 128], F32, tag="ident")
            nc.sync.dma_start(ident[:], di["ident"][:, :])
            WdsT = []
            WrsT = []
            for hl in range(2):
                w = pp.tile([128, 11, 16], BF16, tag=f"Wds{hl}")
                nc.sync.dma_start(w[:], di["Wds16"][hl])
                WdsT.append(w)
                w = pp.tile([128, 32, 16], BF16, tag=f"Wrs{hl}")
                nc.sync.dma_start(w[:], di["Wrs16"][hl])
                WrsT.append(w)

            cc1i = [dr.tile([4096, 512], F32, tag=f"cc1i{c}", name=f"cc1i{c}")
                    for c in range(CH)]
            cc1o = [[dr.tile([1024, 512], F32, tag=f"cc1o{c}_{s}",
                             name=f"cc1o{c}_{s}", addr_space="Shared")
                     for s in range(4)] for c in range(CH)]
            cc2i = [dr.tile([16, 512], F32, tag=f"cc2i{c}", name=f"cc2i{c}")
                    for c in range(CH)]
            cc2o = [dr.tile([16, 512], F32, tag=f"cc2o{c}", name=f"cc2o{c}",
                            addr_space="Shared") for c in range(CH)]
            RG = [list(range(NCORE))]

            def split_to(t_f32, hi_ap, lo_ap):
                nc.scalar.copy(hi_ap, t_f32[:])
                nc.vector.tensor_tensor(lo_ap, t_f32[:], hi_ap, op=OP.subtract)

            for c in range(CH):
                ct = c % 2
                # ================= h1 GEMM (kt-outer) + var1 + split
                var_ps = ps_var.tile([1, 512], F32, tag="var")
                xt = {}
                for kt in range(4):
                    for hl in range(2):
                        t = xp.tile([128, 512], BF16, tag="xp", name=f"x{c}_{kt}_{hl}")
                        nc.sync.dma_start(
                            t[:], di["xT16"][hl, kt, :, c * 512:(c + 1) * 512])
                        xt[kt, hl] = t
                for m in range(32):
                    wt = []
                    for hl in range(2):
                        w = wst.tile([128, 4, 128], BF16, tag="wst")
                        nc.sync.dma_start(w[:], di["Win16"][hl, m])
                        wt.append(w)
                    ps = ps_acc.tile([128, 512], F32, tag="acc")
                    nmm = 0
                    for kt in range(4):
                        for whl, xhl in P3:
                            nc.tensor.matmul(ps[:], wt[whl][:, kt], xt[kt, xhl][:],
                                             start=(nmm == 0), stop=(nmm == 11))
                            nmm += 1
                    t = evp.tile([128, 512], F32, tag="ev")
                    nc.vector.tensor_scalar_add(t[:], ps[:], bin_t[:, m:m + 1])
                    sq = sqp.tile([128, 512], BF16, tag="sq")
                    nc.vector.tensor_tensor(sq[:], t[:], t[:], op=OP.mult)
                    nc.tensor.matmul(var_ps[:], ones16[:], sq[:],
                                     start=(m == 0), stop=(m == 31))
                    split_to(t, H1hi[:, m], H1lo[:, m])

                # ================= s1, s1b, s1T
                u1 = sml.tile([1, 512], F32, tag="sml")
                nc.vector.tensor_scalar(u1[:], var_ps[:], 1.0 / D, EPS,
                                        op0=OP.mult, op1=OP.add)
                r1 = sml.tile([1, 512], F32, tag="sml")
                nc.vector.reciprocal(r1[:], u1[:])
                s1 = sml.tile([1, 512], F32, tag="sml")
                nc.scalar.activation(s1[:], r1[:], AF.Sqrt)
                s1b = bcp.tile([128, 512], F32, tag="bc")
                nc.gpsimd.partition_broadcast(s1b[:], s1[:])
                s1T = pp.tile([128, 4], F32, tag="s1T")
                for t4 in range(4):
                    tp = ps_den.tile([128, 16], F32, tag="den")
                    nc.tensor.transpose(tp[:, 0:1],
                                        s1[0:1, t4 * 128:(t4 + 1) * 128],
                                        ident[0:1, 0:1])
                    nc.vector.tensor_copy(s1T[:, t4:t4 + 1], tp[:, 0:1])

                # ================= q, k GEMMs (kt-outer)
                for which, W16 in (("q", "Wq16"), ("k", "Wk16")):
                    for mh in range(4):
                        ps = ps_acc.tile([128, 512], F32, tag="acc")
                        nmm = 0
                        for qu in range(4):
                            wq = []
                            for hl in range(2):
                                w = wst.tile([128, 8, 128], BF16, tag="wst")
                                nc.sync.dma_start(
                                    w[:], di[W16][hl, mh, :, qu * 8:(qu + 1) * 8])
                                wq.append(w)
                            for k8 in range(8):
                                kt = qu * 8 + k8
                                for whl, xhl in P3:
                                    nc.tensor.matmul(
                                        ps[:], wq[whl][:, k8], HP[xhl][:, kt],
                                        start=(nmm == 0), stop=(nmm == 95))
                                    nmm += 1
                        t = evp.tile([128, 512], F32, tag="ev")
                        nc.vector.tensor_tensor(t[:], ps[:], s1b[:], op=OP.mult)
                        if which == "q":
                            split_to(t, Qhi[:, mh], Qlo[:, mh])
                        else:
                            split_to(t, Khi[:, mh, ct * 512:(ct + 1) * 512],
                                     Klo[:, mh, ct * 512:(ct + 1) * 512])

                # ================= v GEMM (token-major), 2 sweeps
                for sw in range(2):
                    pss = [ps_acc.tile([128, 512], F32, tag="acc", name=f"vps{c}_{sw}_{i}") for i in range(2)]
                    for kt in range(32):
                        wv = []
                        for hl in range(2):
                            w = wst.tile([128, 512], BF16, tag="wst")
                            nc.sync.dma_start(w[:], di["Wv16"][hl, kt])
                            wv.append(w)
                        for i in range(2):
                            t4 = sw * 2 + i
                            hsl = slice(t4 * 128, (t4 + 1) * 128)
                            trio = ((HP[0], wv[0]), (HP[1], wv[0]), (HP[0], wv[1]))
                            for j, (lh, rh) in enumerate(trio):
                                nc.tensor.matmul(
                                    pss[i][:], lh[:, kt, hsl], rh[:],
                                    start=(kt == 0 and j == 0),
                                    stop=(kt == 31 and j == 2))
                    for i in range(2):
                        t4 = sw * 2 + i
                        t = evp.tile([128, 512], F32, tag="ev")
                        nc.vector.tensor_scalar_mul(t[:], pss[i][:],
                                                    s1T[:, t4:t4 + 1])
                        split_to(t, Vhi[:, ct * 4 + t4], Vlo[:, ct * 4 + t4])

                # ================= attention
                njt = 4 * (ct + 1)
                for h in range(4):
                    ctx_ps = ps_ctx.tile([128, 512], F32, tag="ctx")
                    den_ps = ps_var.tile([1, 512], F32, tag="var")
                    hsl = slice(h * 128, (h + 1) * 128)
                    for jt in range(njt):
                        jsl = slice(jt * 128, (jt + 1) * 128)
                        s_ps = ps_acc.tile([128, 512], F32, tag="acc")
                        trio = ((Khi, Qhi), (Klo, Qhi), (Khi, Qlo))
                        for j, (lh, rh) in enumerate(trio):
                            nc.tensor.matmul(s_ps[:], lh[:, h, jsl], rh[:, h],
                                             start=(j == 0), stop=(j == 2))
                        P = ppl.tile([128, 512], F32, tag="pp")
                        nc.scalar.activation(P[:], s_ps[:], AF.Exp)
                        dix = jt - (njt - 4)
                        if dix >= 0:
                            Pm = ppl.tile([128, 512], F32, tag="pp")
                            nc.vector.tensor_tensor(Pm[:], P[:], maskt[:, dix],
                                                    op=OP.mult)
                            P = Pm
                        phi = pbp.tile([128, 512], BF16, tag="pb")
                        nc.scalar.copy(phi[:], P[:])
                        plo = pbp.tile([128, 512], BF16, tag="pb")
                        nc.vector.tensor_tensor(plo[:], P[:], phi[:],
                                                op=OP.subtract)
                        nc.tensor.matmul(den_ps[:], ones16[:], phi[:],
                                         start=(jt == 0), stop=False)
                        nc.tensor.matmul(den_ps[:], ones16[:], plo[:],
                                         start=False, stop=(jt == njt - 1))
                        trc = ((Vhi, phi), (Vlo, phi), (Vhi, plo))
                        for j, (lh, rh) in enumerate(trc):
                            nc.tensor.matmul(ctx_ps[:], lh[:, jt, hsl], rh[:],
                                             start=(jt == 0 and j == 0),
                                             stop=(jt == njt - 1 and j == 2))
                    rec = sml.tile([1, 512], F32, tag="sml")
                    nc.vector.reciprocal(rec[:], den_ps[:])
                    recb = bcp.tile([128, 512], F32, tag="bc")
                    nc.gpsimd.partition_broadcast(recb[:], rec[:])
                    t = evp.tile([128, 512], F32, tag="ev")
                    nc.vector.tensor_tensor(t[:], ctx_ps[:], recb[:], op=OP.mult)
                    split_to(t, CXhi[:, h], CXlo[:, h])

                # ================= Wo + residual/8 + slab AllReduce
                for m in range(32):
                    wt = []
                    for hl in range(2):
                        w = wst.tile([128, 4, 128], BF16, tag="wst")
                        nc.sync.dma_start(w[:], di["Wo16"][hl, m])
                        wt.append(w)
                    ps = ps_acc.tile([128, 512], F32, tag="acc")
                    nmm = 0
                    for cv in range(4):
                        for whl, xhl in P3:
                            nc.tensor.matmul(ps[:], wt[whl][:, cv], CXP[xhl][:, cv],
                                             start=(nmm == 0), stop=(nmm == 11))
                            nmm += 1
                    a1 = evp.tile([128, 512], F32, tag="ev")
                    nc.vector.scalar_tensor_tensor(a1[:], H1hi[:, m], 0.125, ps[:],
                                                   op0=OP.mult, op1=OP.add)
                    a2 = evp.tile([128, 512], F32, tag="ev")
                    nc.vector.scalar_tensor_tensor(a2[:], H1lo[:, m], 0.125, a1[:],
                                                   op0=OP.mult, op1=OP.add)
                    nc.scalar.dma_start(cc1i[c][m * 128:(m + 1) * 128, :], a2[:])
                    if m % 8 == 7:
                        sl = slice((m // 8) * 1024, (m // 8 + 1) * 1024)
                        nc.gpsimd.collective_compute(
                            "AllReduce", OP.add, replica_groups=RG,
                            ins=[cc1i[c][sl, :].opt()],
                            outs=[cc1o[c][m // 8][:].opt()])

                # ================= h2 load + var2 + split (same H buffers)
                var2_ps = ps_var.tile([1, 512], F32, tag="var")
                for m in range(32):
                    t = h2l.tile([128, 512], F32, tag="h2l")
                    nc.sync.dma_start(
                        t[:], cc1o[c][m // 8][(m % 8) * 128:(m % 8 + 1) * 128, :])
                    sq = sqp.tile([128, 512], BF16, tag="sq")
                    nc.vector.tensor_tensor(sq[:], t[:], t[:], op=OP.mult)
                    nc.tensor.matmul(var2_ps[:], ones16[:], sq[:],
                                     start=(m == 0), stop=(m == 31))
                    split_to(t, H1hi[:, m], H1lo[:, m])
                u2 = sml.tile([1, 512], F32, tag="sml")
                nc.vector.tensor_scalar(u2[:], var2_ps[:], 1.0 / D, EPS,
                                        op0=OP.mult, op1=OP.add)
                r2 = sml.tile([1, 512], F32, tag="sml")
                nc.vector.reciprocal(r2[:], u2[:])
                s2 = sml.tile([1, 512], F32, tag="sml")
                nc.scalar.activation(s2[:], r2[:], AF.Sqrt)
                s2b = bcp.tile([128, 512], F32, tag="bc")
                nc.gpsimd.partition_broadcast(s2b[:], s2[:])

                # ================= MLP (kt-outer) + rl partials
                rl_ps = ps_rl.tile([16, 512], F32, tag="rl")
                for f in range(11):
                    for gi, W16 in enumerate(("Wg16", "Wu16")):
                        ps = ps_acc.tile([128, 512], F32, tag="acc")
                        nmm = 0
                        for qu in range(4):
                            wq = []
                            for hl in range(2):
                                w = wgu.tile([128, 8, 128], BF16, tag="wgu")
                                nc.sync.dma_start(
                                    w[:], di[W16][hl, f, :, qu * 8:(qu + 1) * 8])
                                wq.append(w)
                            for k8 in range(8):
                                kt = qu * 8 + k8
                                for whl, xhl in P3:
                                    nc.tensor.matmul(
                                        ps[:], wq[whl][:, k8], HP[xhl][:, kt],
                                        start=(nmm == 0), stop=(nmm == 95))
                                    nmm += 1
                        if gi == 0:
                            gps = ps
                        else:
                            ups = ps
                    gt = evp.tile([128, 512], F32, tag="ev")
                    nc.vector.tensor_tensor(gt[:], gps[:], s2b[:], op=OP.mult)
                    gs = evp.tile([128, 512], F32, tag="ev")
                    nc.scalar.activation(gs[:], gt[:], AF.Silu)
                    ut = evp.tile([128, 512], F32, tag="ev")
                    nc.vector.tensor_tensor(ut[:], ups[:], s2b[:], op=OP.mult)
                    at = evp.tile([128, 512], F32, tag="ev")
                    nc.vector.tensor_tensor(at[:], gs[:], ut[:], op=OP.mult)
                    ahi = pbp.tile([128, 512], BF16, tag="pb")
                    nc.scalar.copy(ahi[:], at[:])
                    alo = pbp.tile([128, 512], BF16, tag="pb")
                    nc.vector.tensor_tensor(alo[:], at[:], ahi[:], op=OP.subtract)
                    trr = ((WdsT[0], ahi), (WdsT[1], ahi), (WdsT[0], alo))
                    for j, (lh, rh) in enumerate(trr):
                        nc.tensor.matmul(rl_ps[:], lh[:, f], rh[:],
                                         start=(f == 0 and j == 0),
                                         stop=(f == 10 and j == 2))
                rlt = rlp16.tile([16, 512], F32, tag="rl16")
                nc.vector.tensor_copy(rlt[:], rl_ps[:])
                nc.scalar.dma_start(cc2i[c][:, :], rlt[:])
                nc.gpsimd.collective_compute(
                    "AllReduce", OP.add, replica_groups=RG,
                    ins=[cc2i[c][:].opt()], outs=[cc2o[c][:].opt()])

                # ================= final stage (replicated on all cores)
                rlo_ps = ps_rl.tile([16, 512], F32, tag="rl")
                nmm = 0
                for kt in range(32):
                    for whl, xhl in P3:
                        nc.tensor.matmul(rlo_ps[:], WrsT[whl][:, kt], HP[xhl][:, kt],
                                         start=(nmm == 0), stop=(nmm == 95))
                        nmm += 1
                mlp16 = rlp16.tile([16, 512], F32, tag="rl16")
                nc.sync.dma_start(mlp16[:], cc2o[c][:])
                rl16 = rlp16.tile([16, 512], F32, tag="rlf")
                nc.vector.scalar_tensor_tensor(rl16[:], rlo_ps[:], br16[:, 0:1],
                                               mlp16[:], op0=OP.add, op1=OP.add)
                for t4 in range(4):
                    tp = ps_den.tile([128, 16], F32, tag="den")
                    nc.tensor.transpose(tp[:, 0:16],
                                        rl16[:, t4 * 128:(t4 + 1) * 128],
                                        ident[0:16, 0:16])
                    rt = fin.tile([128, 16], F32, tag="fin")
                    nc.vector.tensor_copy(rt[:], tp[:, 0:16])
                    rl8 = rt[:, 0:8]
                    h8 = rt[:, 8:16]
                    m1 = fin.tile([128, 1], F32, tag="fin1")
                    nc.vector.tensor_reduce(m1[:], rl8, AX.X, OP.max)
                    eq1 = fin.tile([128, 8], dt.int32, tag="fini")
                    nc.vector.tensor_scalar(eq1[:], rl8, m1[:], None,
                                            op0=OP.is_equal)
                    cand = fin.tile([128, 8], F32, tag="fin")
                    nc.vector.select(cand[:], eq1[:], iota8b[:], c99[:])
                    idx1 = fin.tile([128, 1], F32, tag="fin1")
                    nc.vector.tensor_reduce(idx1[:], cand[:], AX.X, OP.min)
                    eqi1 = fin.tile([128, 8], dt.int32, tag="fini")
                    nc.vector.tensor_scalar(eqi1[:], iota8b[:], idx1[:], None,
                                            op0=OP.is_equal)
                    sel1 = fin.tile([128, 8], F32, tag="fin")
                    nc.vector.select(sel1[:], eqi1[:], h8, zero8[:])
                    v1 = fin.tile([128, 1], F32, tag="fin1")
                    nc.vector.tensor_reduce(v1[:], sel1[:], AX.X, OP.add)
                    rl8b = fin.tile([128, 8], F32, tag="fin")
                    nc.vector.select(rl8b[:], eqi1[:], negb[:], rl8)
                    m2 = fin.tile([128, 1], F32, tag="fin1")
                    nc.vector.tensor_reduce(m2[:], rl8b[:], AX.X, OP.max)
                    eq2 = fin.tile([128, 8], dt.int32, tag="fini")
                    nc.vector.tensor_scalar(eq2[:], rl8b[:], m2[:], None,
                                            op0=OP.is_equal)
                    cand2 = fin.tile([128, 8], F32, tag="fin")
                    nc.vector.select(cand2[:], eq2[:], iota8b[:], c99[:])
                    idx2 = fin.tile([128, 1], F32, tag="fin1")
                    nc.vector.tensor_reduce(idx2[:], cand2[:], AX.X, OP.min)
                    eqi2 = fin.tile([128, 8], dt.int32, tag="fini")
                    nc.vector.tensor_scalar(eqi2[:], iota8b[:], idx2[:], None,
                                            op0=OP.is_equal)
                    sel2 = fin.tile([128, 8], F32, tag="fin")
                    nc.vector.select(sel2[:], eqi2[:], h8, zero8[:])
                    v2 = fin.tile([128, 1], F32, tag="fin1")
                    nc.vector.tensor_reduce(v2[:], sel2[:], AX.X, OP.add)
                    agg = fin.tile([128, 1], F32, tag="fin1")
                    nc.vector.tensor_tensor(agg[:], v1[:], v2[:], op=OP.add)
                    outt = otp.tile([128, 512], F32, tag="ot")
                    nc.vector.scalar_tensor_tensor(outt[:], wsumb[:], agg[:],
                                                   boutb[:], op0=OP.mult,
                                                   op1=OP.add)
                    nc.gpsimd.dma_start(
                        out_d[c * 512 + t4 * 128: c * 512 + (t4 + 1) * 128, :],
                        outt[:])
    nc.compile()
    return nc


def _prepare_inputs(inputs):
    f32 = np.float32
    inp = {k: np.asarray(v, f32) for k, v in inputs.items()}
    ln1, ln2 = inp["ln1_w"], inp["ln2_w"]
    Wq_f = ln1[:, None] * inp["Wq"]
    Wk_f = ln1[:, None] * inp["Wk"] / np.sqrt(DH)
    Wv_f = ln1[:, None] * inp["Wv"]
    Wg_f = np.zeros((D, FFP), f32); Wg_f[:, :FF] = ln2[:, None] * inp["W_gate"]
    Wu_f = np.zeros((D, FFP), f32); Wu_f[:, :FF] = ln2[:, None] * inp["W_up"]
    Wds = np.zeros((FFP, 16), f32)
    Wds[:FF, :8] = inp["W_down"] @ inp["W_router"]
    Wds[:FF, 8:] = inp["W_down"][:, :8]
    Wrs = np.zeros((D, 16), f32)
    Wrs[:, :8] = inp["W_router"]; Wrs[:8, 8:] = np.eye(8, dtype=f32)
    wsum = 0.5 * inp["W_out"].sum(0)

    xT = inp["x"].reshape(NT, DIN).T.copy()
    masks = np.zeros((4, 128, 512), f32)
    jj = np.arange(128)[:, None]; ii = np.arange(512)[None, :]
    for dx in range(4):
        masks[dx] = (jj + dx * 128 <= ii)

    def c(a):
        return np.ascontiguousarray(a)

    common = {
        "xT16": c(_split(xT.reshape(4, 128, NT))),
        "Win16": c(_split(inp["W_in"].reshape(4, 128, 32, 128)
                          .transpose(2, 1, 0, 3))),
        "masks16": c(masks.transpose(1, 0, 2).astype(BF)),
        "bin_t": c(inp["b_in"].reshape(32, 128).T),
        "br16": np.concatenate([inp["b_router"],
                                np.zeros(8, f32)])[:, None].copy(),
        "wsumb": c(np.tile(wsum[None, :], (128, 1))),
        "boutb": c(np.tile(inp["b_out"][None, :], (128, 1))),
        "iota8b": c(np.tile(np.arange(8, dtype=f32)[None, :], (128, 1))),
        "ident": np.eye(128, dtype=f32),
    }
    in_maps = []
    for r in range(NCORE):
        hs = slice(r * HPC * DH, (r + 1) * HPC * DH)
        fs = slice(r * FFS, (r + 1) * FFS)
        m = dict(common)
        m["Wq16"] = c(_split(Wq_f[:, hs].reshape(32, 128, 4, 128)
                             .transpose(2, 1, 0, 3)))
        m["Wk16"] = c(_split(Wk_f[:, hs].reshape(32, 128, 4, 128)
                             .transpose(2, 1, 0, 3)))
        m["Wv16"] = c(_split(Wv_f[:, hs].reshape(32, 128, 512)))
        m["Wo16"] = c(_split(inp["Wo"][hs, :].reshape(4, 128, 32, 128)
                             .transpose(2, 1, 0, 3)))
        m["Wg16"] = c(_split(Wg_f[:, fs].reshape(32, 128, 11, 128)
                             .transpose(2, 1, 0, 3)))
        m["Wu16"] = c(_split(Wu_f[:, fs].reshape(32, 128, 11, 128)
                             .transpose(2, 1, 0, 3)))
        m["Wds16"] = c(_split(Wds[fs, :].reshape(11, 128, 16)
                              .transpose(1, 0, 2)))
        m["Wrs16"] = c(_split(Wrs.reshape(32, 128, 16).transpose(1, 0, 2)))
        in_maps.append(m)
    return in_maps


_NC_CACHE = None


def kernel(**inputs):
    global LAST_RESULT, _NC_CACHE
    if _NC_CACHE is None:
        _NC_CACHE = _build()
    in_maps = _prepare_inputs(inputs)
    res = run_bass_kernel_spmd(_NC_CACHE, in_maps, core_ids=list(range(NCORE)))
    LAST_RESULT = res
    return res.results[0]["out"].reshape(B, S, 512).astype(np.float32)


# revision 9
# speedup vs baseline: 1.0138x; 1.0138x over previous
"""TRN2 Bass kernel for nn_DeepSeekPretrainedMoE (8-core tensor-parallel).

Algorithm (validated vs reference in numpy mirror, l2 rel ~1.4e-6):
  h1 = x@W_in + b_in; hn1 = rmsnorm(h1)*ln1  (ln1 folded into Wq/Wk/Wv)
  attention (4 heads/core, causal, softmax without max-subtraction),
  h2 = AllReduce(ctx@Wo_shard + h1/8); hn2 = rmsnorm(h2)*ln2 (folded)
  act = silu(hn2@Wg_shard) * (hn2@Wu_shard)      [FF column-sharded]
  rl16 = h2@[W_router|Sel8] + AllReduce(act@[W_down@W_router|W_down[:,:8]])
  top-2 of rl16[:8] -> gather rl16[8:] -> agg; out = agg*0.5*colsum(W_out)+b_out

Precision: all router-critical GEMMs in bf16 hi/lo 3-pass (err ~1e-5 rel).
Layout: feature-major activations [D, tokens]; 4 token-chunks of 512.
"""
import contextlib
import ctypes
import sys
import types

sys.path.insert(0, "/opt/trn_rl_repo")

import numpy as np
import ml_dtypes


def _install_ntff_hook():
    if "antenv.axon_hooks" in sys.modules:
        return
    hook = None
    try:
        lib = ctypes.CDLL("/opt/axon/libaxon_pjrt.so")
        if hasattr(lib, "axon_start_nrt_profile"):
            lib.axon_start_nrt_profile.argtypes = [
                ctypes.POINTER(ctypes.c_int64), ctypes.c_size_t]
            lib.axon_start_nrt_profile.restype = ctypes.c_int64
            lib.axon_stop_nrt_profile.argtypes = [ctypes.c_char_p]
            lib.axon_stop_nrt_profile.restype = ctypes.c_int64

            @contextlib.contextmanager
            def hook(output_dir, device_ids):
                import jax
                jax.devices()
                if device_ids:
                    ids = (ctypes.c_int64 * len(device_ids))(*device_ids)
                    rc = lib.axon_start_nrt_profile(ids, len(device_ids))
                else:
                    rc = lib.axon_start_nrt_profile(None, 0)
                if rc != 0:
                    raise RuntimeError(f"axon_start_nrt_profile rc={rc}")
                try:
                    yield
                finally:
                    n = lib.axon_stop_nrt_profile(str(output_dir).encode())
                    if n < 0:
                        raise RuntimeError(f"axon_stop_nrt_profile rc={n}")
    except OSError:
        pass
    mod = types.ModuleType("antenv.axon_hooks")
    mod.get_axon_ntff_profile_hook = lambda: hook

    def _set(h):
        mod.get_axon_ntff_profile_hook = lambda: h
    mod.set_axon_ntff_profile_hook = _set
    import antenv
    antenv.axon_hooks = mod
    sys.modules["antenv.axon_hooks"] = mod


_install_ntff_hook()

import concourse.bacc as bacc            # noqa: E402
import concourse.mybir as mybir          # noqa: E402
import concourse.tile as tile            # noqa: E402
from concourse.bass_utils import run_bass_kernel_spmd  # noqa: E402
import concourse.bass_utils as _bu                      # noqa: E402

_orig_run_command = _bu.run_command


def _run_command_ldwopt(argv, **kw):
    argv = ["--enable-ldw-opt=true" if a == "--enable-ldw-opt=false" else a
            for a in argv]
    return _orig_run_command(argv, **kw)


_bu.run_command = _run_command_ldwopt
from concourse.alu_op_type import AluOpType as OP      # noqa: E402
import bass_rust                          # noqa: E402

AF = bass_rust.ActivationFunctionType
AX = mybir.AxisListType
dt = mybir.dt
F32, BF16 = dt.float32, dt.bfloat16

B, S, DIN, D, H, DH, FF, E = 2, 1024, 512, 4096, 32, 128, 11008, 8
NCORE, HPC = 8, 4
FFP, FFS = 11264, 1408
NT = B * S
CH = 4
EPS = 1e-6
BF = ml_dtypes.bfloat16
P3 = ((0, 0), (0, 1), (1, 0))   # (w_half, x_half); consecutive share Whi

LAST_RESULT = None


def _split(a):
    hi = a.astype(BF)
    lo = (a.astype(np.float32) - hi.astype(np.float32)).astype(BF)
    return np.stack([hi, lo])


def _build():
    nc = bacc.Bacc("TRN2", target_bir_lowering=False)
    di = {}

    def inp(name, shape, d=BF16):
        di[name] = nc.dram_tensor(name, shape, d, kind="ExternalInput")

    inp("xT16", [2, 4, 128, NT])
    inp("Win16", [2, 32, 128, 4, 128])
    inp("Wq16", [2, 4, 128, 32, 128])
    inp("Wk16", [2, 4, 128, 32, 128])
    inp("Wv16", [2, 32, 128, 512])
    inp("Wo16", [2, 32, 128, 4, 128])
    inp("Wg16", [2, 11, 128, 32, 128])
    inp("Wu16", [2, 11, 128, 32, 128])
    inp("Wds16", [2, 128, 11, 16])
    inp("Wrs16", [2, 128, 32, 16])
    inp("masks16", [128, 4, 512])
    inp("bin_t", [128, 32], F32)
    inp("br16", [16, 1], F32)
    inp("wsumb", [128, 512], F32)
    inp("boutb", [128, 512], F32)
    inp("iota8b", [128, 8], F32)
    inp("ident", [128, 128], F32)
    out_d = nc.dram_tensor("out", [NT, 512], F32, kind="ExternalOutput")

    with contextlib.ExitStack() as _st:
        tc = _st.enter_context(tile.TileContext(nc))
        ec = _st.enter_context
        pp = ec(tc.tile_pool(name="persist", bufs=1))
        wst = ec(tc.tile_pool(name="wst", bufs=6))
        xp = ec(tc.tile_pool(name="xp", bufs=8))
        wgu = ec(tc.tile_pool(name="wgu", bufs=8))
        evp = ec(tc.tile_pool(name="ev", bufs=4))
        sqp = ec(tc.tile_pool(name="sqp", bufs=2))
        ppl = ec(tc.tile_pool(name="ppool", bufs=2))
        pbp = ec(tc.tile_pool(name="pb", bufs=4))
        sml = ec(tc.tile_pool(name="sml", bufs=3))
        rlp16 = ec(tc.tile_pool(name="rl16p", bufs=2))
        bcp = ec(tc.tile_pool(name="bc", bufs=2))
        fin = ec(tc.tile_pool(name="fin", bufs=10))
        otp = ec(tc.tile_pool(name="ot", bufs=2))
        h2l = ec(tc.tile_pool(name="h2l", bufs=2))
        ps_acc = ec(tc.tile_pool(name="ps_acc", bufs=4, space="PSUM"))
        ps_ctx = ec(tc.tile_pool(name="ps_ctx", bufs=1, space="PSUM"))
        ps_den = ec(tc.tile_pool(name="ps_den", bufs=1, space="PSUM"))
        ps_var = ec(tc.tile_pool(name="ps_var", bufs=1, space="PSUM"))
        ps_rl = ec(tc.tile_pool(name="ps_rl", bufs=1, space="PSUM"))
        dr = ec(tc.tile_pool(name="dram", bufs=1, space="DRAM"))
        if True:
            H1hi = pp.tile([128, 32, 512], BF16, tag="H1hi")
            H1lo = pp.tile([128, 32, 512], BF16, tag="H1lo")
            HP = (H1hi, H1lo)
            Khi = pp.tile([128, 4, 1024], BF16, tag="Khi")
            Klo = pp.tile([128, 4, 1024], BF16, tag="Klo")
            Vhi = pp.tile([128, 8, 512], BF16, tag="Vhi")
            Vlo = pp.tile([128, 8, 512], BF16, tag="Vlo")
            Qhi = pp.tile([128, 4, 512], BF16, tag="Qhi")
            Qlo = pp.tile([128, 4, 512], BF16, tag="Qlo")
            QP = (Qhi, Qlo)
            CXhi = pp.tile([128, 4, 512], BF16, tag="CXhi")
            CXlo = pp.tile([128, 4, 512], BF16, tag="CXlo")
            CXP = (CXhi, CXlo)
            ones16 = pp.tile([128, 1], BF16, tag="ones16")
            nc.vector.memset(ones16[:], 1.0)
            c99 = pp.tile([128, 8], F32, tag="c99")
            nc.vector.memset(c99[:], 99.0)
            negb = pp.tile([128, 8], F32, tag="negb")
            nc.vector.memset(negb[:], -1e30)
            zero8 = pp.tile([128, 8], F32, tag="zero8")
            nc.vector.memset(zero8[:], 0.0)
            maskt = pp.tile([128, 4, 512], BF16, tag="maskt")
            nc.sync.dma_start(maskt[:], di["masks16"][:, :, :])
            bin_t = pp.tile([128, 32], F32, tag="bin_t")
            nc.sync.dma_start(bin_t[:], di["bin_t"][:, :])
            br16 = pp.tile([16, 1], F32, tag="br16")
            nc.sync.dma_start(br16[:], di["br16"][:, :])
            wsumb = pp.tile([128, 512], F32, tag="wsumb")
            nc.sync.dma_start(wsumb[:], di["wsumb"][:, :])
            boutb = pp.tile([128, 512], F32, tag="boutb")
            nc.sync.dma_start(boutb[:], di["boutb"][:, :])
            iota8b = pp.tile([128, 8], F32, tag="iota8b")
            nc.sync.dma_start(iota8b[:], di["iota8b"][:, :])
            ident = pp.tile([128, 128], F32, tag="ident")
            nc.sync.dma_start(ident[:], di["ident"][:, :])
            WdsT = []
            WrsT = []
            for hl in range(2):
                w = pp.tile([128, 11, 16], BF16, tag=f"Wds{hl}")
                nc.sync.dma_start(w[:], di["Wds16"][hl])
                WdsT.append(w)
                w = pp.tile([128, 32, 16], BF16, tag=f"Wrs{hl}")
                nc.sync.dma_start(w[:], di["Wrs16"][hl])
                WrsT.append(w)

            cc1i = [dr.tile([4096, 512], F32, tag=f"cc1i{c}", name=f"cc1i{c}")
                    for c in range(CH)]
            cc1o = [[dr.tile([1024, 512], F32, tag=f"cc1o{c}_{s}",
                             name=f"cc1o{c}_{s}", addr_space="Shared")
                     for s in range(4)] for c in range(CH)]
            cc2i = [dr.tile([16, 512], F32, tag=f"cc2i{c}", name=f"cc2i{c}")
                    for c in range(CH)]
            cc2o = [dr.tile([16, 512], F32, tag=f"cc2o{c}", name=f"cc2o{c}",
                            addr_space="Shared") for c in range(CH)]
            RG = [list(range(NCORE))]

            def split_to(t_f32, hi_ap, lo_ap):
                nc.scalar.copy(hi_ap, t_f32[:])
                nc.vector.tensor_tensor(lo_ap, t_f32[:], hi_ap, op=OP.subtract)

            for c in range(CH):
                ct = c % 2
                # ================= h1 GEMM (kt-outer) + var1 + split
                var_ps = ps_var.tile([1, 512], F32, tag="var")
                xt = {}
                for kt in range(4):
                    for hl in range(2):
                        t = xp.tile([128, 512], BF16, tag="xp", name=f"x{c}_{kt}_{hl}")
                        nc.sync.dma_start(
                            t[:], di["xT16"][hl, kt, :, c * 512:(c + 1) * 512])
                        xt[kt, hl] = t
                for m in range(32):
                    wt = []
                    for hl in range(2):
                        w = wst.tile([128, 4, 128], BF16, tag="wst")
                        nc.sync.dma_start(w[:], di["Win16"][hl, m])
                        wt.append(w)
                    ps = ps_acc.tile([128, 512], F32, tag="acc")
                    nmm = 0
                    for kt in range(4):
                        for whl, xhl in P3:
                            nc.tensor.matmul(ps[:], wt[whl][:, kt], xt[kt, xhl][:],
                                             start=(nmm == 0), stop=(nmm == 11))
                            nmm += 1
                    t = evp.tile([128, 512], F32, tag="ev")
                    nc.vector.tensor_scalar_add(t[:], ps[:], bin_t[:, m:m + 1])
                    sq = sqp.tile([128, 512], BF16, tag="sq")
                    nc.vector.tensor_tensor(sq[:], t[:], t[:], op=OP.mult)
                    nc.tensor.matmul(var_ps[:], ones16[:], sq[:],
                                     start=(m == 0), stop=(m == 31))
                    split_to(t, H1hi[:, m], H1lo[:, m])

                # ================= s1, s1b, s1T
                u1 = sml.tile([1, 512], F32, tag="sml")
                nc.vector.tensor_scalar(u1[:], var_ps[:], 1.0 / D, EPS,
                                        op0=OP.mult, op1=OP.add)
                r1 = sml.tile([1, 512], F32, tag="sml")
                nc.vector.reciprocal(r1[:], u1[:])
                s1 = sml.tile([1, 512], F32, tag="sml")
                nc.scalar.activation(s1[:], r1[:], AF.Sqrt)
                s1b = bcp.tile([128, 512], F32, tag="bc")
                nc.gpsimd.partition_broadcast(s1b[:], s1[:])
                s1T = pp.tile([128, 4], F32, tag="s1T")
                for t4 in range(4):
                    tp = ps_den.tile([128, 16], F32, tag="den")
                    nc.tensor.transpose(tp[:, 0:1],
                                        s1[0:1, t4 * 128:(t4 + 1) * 128],
                                        ident[0:1, 0:1])
                    nc.vector.tensor_copy(s1T[:, t4:t4 + 1], tp[:, 0:1])

                # ================= q, k GEMMs (kt-outer)
                for which, W16 in (("q", "Wq16"), ("k", "Wk16")):
                    for mh in range(4):
                        ps = ps_acc.tile([128, 512], F32, tag="acc")
                        nmm = 0
                        for qu in range(4):
                            wq = []
                            for hl in range(2):
                                w = wst.tile([128, 8, 128], BF16, tag="wst")
                                nc.sync.dma_start(
                                    w[:], di[W16][hl, mh, :, qu * 8:(qu + 1) * 8])
                                wq.append(w)
                            for k8 in range(8):
                                kt = qu * 8 + k8
                                for whl, xhl in P3:
                                    nc.tensor.matmul(
                                        ps[:], wq[whl][:, k8], HP[xhl][:, kt],
                                        start=(nmm == 0), stop=(nmm == 95))
                                    nmm += 1
                        t = evp.tile([128, 512], F32, tag="ev")
                        nc.vector.tensor_tensor(t[:], ps[:], s1b[:], op=OP.mult)
                        if which == "q":
                            split_to(t, Qhi[:, mh], Qlo[:, mh])
                        else:
                            split_to(t, Khi[:, mh, ct * 512:(ct + 1) * 512],
                                     Klo[:, mh, ct * 512:(ct + 1) * 512])

                # ================= v GEMM (token-major), 2 sweeps
                for sw in range(2):
                    pss = [ps_acc.tile([128, 512], F32, tag="acc", name=f"vps{c}_{sw}_{i}") for i in range(2)]
                    for kt in range(32):
                        wv = []
                        for hl in range(2):
                            w = wst.tile([128, 512], BF16, tag="wst")
                            nc.sync.dma_start(w[:], di["Wv16"][hl, kt])
                            wv.append(w)
                        for i in range(2):
                            t4 = sw * 2 + i
                            hsl = slice(t4 * 128, (t4 + 1) * 128)
                            trio = ((HP[0], wv[0]), (HP[1], wv[0]), (HP[0], wv[1]))  # lhsT differs each
                            for j, (lh, rh) in enumerate(trio):
                                nc.tensor.matmul(
                                    pss[i][:], lh[:, kt, hsl], rh[:],
                                    start=(kt == 0 and j == 0),
                                    stop=(kt == 31 and j == 2))
                    for i in range(2):
                        t4 = sw * 2 + i
                        t = evp.tile([128, 512], F32, tag="ev")
                        nc.vector.tensor_scalar_mul(t[:], pss[i][:],
                                                    s1T[:, t4:t4 + 1])
                        split_to(t, Vhi[:, ct * 4 + t4], Vlo[:, ct * 4 + t4])

                # ================= attention
                njt = 4 * (ct + 1)
                for h in range(4):
                    ctx_ps = ps_ctx.tile([128, 512], F32, tag="ctx")
                    den_ps = ps_var.tile([1, 512], F32, tag="var")
                    hsl = slice(h * 128, (h + 1) * 128)
                    for jt in range(njt):
                        jsl = slice(jt * 128, (jt + 1) * 128)
                        s_ps = ps_acc.tile([128, 512], F32, tag="acc")
                        trio = ((Khi, Qhi), (Khi, Qlo), (Klo, Qhi))
                        for j, (lh, rh) in enumerate(trio):
                            nc.tensor.matmul(s_ps[:], lh[:, h, jsl], rh[:, h],
                                             start=(j == 0), stop=(j == 2))
                        P = ppl.tile([128, 512], F32, tag="pp")
                        nc.scalar.activation(P[:], s_ps[:], AF.Exp)
                        dix = jt - (njt - 4)
                        if dix >= 0:
                            Pm = ppl.tile([128, 512], F32, tag="pp")
                            nc.vector.tensor_tensor(Pm[:], P[:], maskt[:, dix],
                                                    op=OP.mult)
                            P = Pm
                        phi = pbp.tile([128, 512], BF16, tag="pb")
                        nc.scalar.copy(phi[:], P[:])
                        plo = pbp.tile([128, 512], BF16, tag="pb")
                        nc.vector.tensor_tensor(plo[:], P[:], phi[:],
                                                op=OP.subtract)
                        nc.tensor.matmul(den_ps[:], ones16[:], phi[:],
                                         start=(jt == 0), stop=False)
                        nc.tensor.matmul(den_ps[:], ones16[:], plo[:],
                                         start=False, stop=(jt == njt - 1))
                        trc = ((Vhi, phi), (Vhi, plo), (Vlo, phi))
                        for j, (lh, rh) in enumerate(trc):
                            nc.tensor.matmul(ctx_ps[:], lh[:, jt, hsl], rh[:],
                                             start=(jt == 0 and j == 0),
                                             stop=(jt == njt - 1 and j == 2))
                    rec = sml.tile([1, 512], F32, tag="sml")
                    nc.vector.reciprocal(rec[:], den_ps[:])
                    recb = bcp.tile([128, 512], F32, tag="bc")
                    nc.gpsimd.partition_broadcast(recb[:], rec[:])
                    t = evp.tile([128, 512], F32, tag="ev")
                    nc.vector.tensor_tensor(t[:], ctx_ps[:], recb[:], op=OP.mult)
                    split_to(t, CXhi[:, h], CXlo[:, h])

                # ================= Wo + residual/8 + slab AllReduce
                for m in range(32):
                    wt = []
                    for hl in range(2):
                        w = wst.tile([128, 4, 128], BF16, tag="wst")
                        nc.sync.dma_start(w[:], di["Wo16"][hl, m])
                        wt.append(w)
                    ps = ps_acc.tile([128, 512], F32, tag="acc")
                    nmm = 0
                    for cv in range(4):
                        for whl, xhl in P3:
                            nc.tensor.matmul(ps[:], wt[whl][:, cv], CXP[xhl][:, cv],
                                             start=(nmm == 0), stop=(nmm == 11))
                            nmm += 1
                    a1 = evp.tile([128, 512], F32, tag="ev")
                    nc.vector.scalar_tensor_tensor(a1[:], H1hi[:, m], 0.125, ps[:],
                                                   op0=OP.mult, op1=OP.add)
                    a2 = evp.tile([128, 512], F32, tag="ev")
                    nc.vector.scalar_tensor_tensor(a2[:], H1lo[:, m], 0.125, a1[:],
                                                   op0=OP.mult, op1=OP.add)
                    nc.scalar.dma_start(cc1i[c][m * 128:(m + 1) * 128, :], a2[:])
                    if m % 8 == 7:
                        sl = slice((m // 8) * 1024, (m // 8 + 1) * 1024)
                        nc.gpsimd.collective_compute(
                            "AllReduce", OP.add, replica_groups=RG,
                            ins=[cc1i[c][sl, :].opt()],
                            outs=[cc1o[c][m // 8][:].opt()])

                # ================= h2 load + var2 + split (same H buffers)
                var2_ps = ps_var.tile([1, 512], F32, tag="var")
                for m in range(32):
                    t = h2l.tile([128, 512], F32, tag="h2l")
                    nc.sync.dma_start(
                        t[:], cc1o[c][m // 8][(m % 8) * 128:(m % 8 + 1) * 128, :])
                    sq = sqp.tile([128, 512], BF16, tag="sq")
                    nc.vector.tensor_tensor(sq[:], t[:], t[:], op=OP.mult)
                    nc.tensor.matmul(var2_ps[:], ones16[:], sq[:],
                                     start=(m == 0), stop=(m == 31))
                    split_to(t, H1hi[:, m], H1lo[:, m])
                u2 = sml.tile([1, 512], F32, tag="sml")
                nc.vector.tensor_scalar(u2[:], var2_ps[:], 1.0 / D, EPS,
                                        op0=OP.mult, op1=OP.add)
                r2 = sml.tile([1, 512], F32, tag="sml")
                nc.vector.reciprocal(r2[:], u2[:])
                s2 = sml.tile([1, 512], F32, tag="sml")
                nc.scalar.activation(s2[:], r2[:], AF.Sqrt)
                s2b = bcp.tile([128, 512], F32, tag="bc")
                nc.gpsimd.partition_broadcast(s2b[:], s2[:])

                # ================= MLP (kt-outer) + rl partials
                rl_ps = ps_rl.tile([16, 512], F32, tag="rl")
                for f in range(11):
                    for gi, W16 in enumerate(("Wg16", "Wu16")):
                        ps = ps_acc.tile([128, 512], F32, tag="acc")
                        nmm = 0
                        for qu in range(4):
                            wq = []
                            for hl in range(2):
                                w = wgu.tile([128, 8, 128], BF16, tag="wgu")
                                nc.scalar.dma_start(
                                    w[:], di[W16][hl, f, :, qu * 8:(qu + 1) * 8])
                                wq.append(w)
                            for k8 in range(8):
                                kt = qu * 8 + k8
                                for whl, xhl in P3:
                                    nc.tensor.matmul(
                                        ps[:], wq[whl][:, k8], HP[xhl][:, kt],
                                        start=(nmm == 0), stop=(nmm == 95))
                                    nmm += 1
                        if gi == 0:
                            gps = ps
                        else:
                            ups = ps
                    gt = evp.tile([128, 512], F32, tag="ev")
                    nc.vector.tensor_tensor(gt[:], gps[:], s2b[:], op=OP.mult)
                    gs = evp.tile([128, 512], F32, tag="ev")
                    nc.scalar.activation(gs[:], gt[:], AF.Silu)
                    ut = evp.tile([128, 512], F32, tag="ev")
                    nc.vector.tensor_tensor(ut[:], ups[:], s2b[:], op=OP.mult)
                    at = evp.tile([128, 512], F32, tag="ev")
                    nc.vector.tensor_tensor(at[:], gs[:], ut[:], op=OP.mult)
                    ahi = pbp.tile([128, 512], BF16, tag="pb")
                    nc.scalar.copy(ahi[:], at[:])
                    alo = pbp.tile([128, 512], BF16, tag="pb")
                    nc.vector.tensor_tensor(alo[:], at[:], ahi[:], op=OP.subtract)
                    trr = ((WdsT[0], ahi), (WdsT[0], alo), (WdsT[1], ahi))
                    for j, (lh, rh) in enumerate(trr):
                        nc.tensor.matmul(rl_ps[:], lh[:, f], rh[:],
                                         start=(f == 0 and j == 0),
                                         stop=(f == 10 and j == 2))
                rlt = rlp16.tile([16, 512], F32, tag="rl16")
                nc.vector.tensor_copy(rlt[:], rl_ps[:])
                nc.scalar.dma_start(cc2i[c][:, :], rlt[:])
                nc.gpsimd.collective_compute(
                    "AllReduce", OP.add, replica_groups=RG,
                    ins=[cc2i[c][:].opt()], outs=[cc2o[c][:].opt()])

                # ================= final stage (replicated on all cores)
                rlo_ps = ps_rl.tile([16, 512], F32, tag="rl")
                nmm = 0
                for kt in range(32):
                    for whl, xhl in P3:
                        nc.tensor.matmul(rlo_ps[:], WrsT[whl][:, kt], HP[xhl][:, kt],
                                         start=(nmm == 0), stop=(nmm == 95))
                        nmm += 1
                mlp16 = rlp16.tile([16, 512], F32, tag="rl16")
                nc.sync.dma_start(mlp16[:], cc2o[c][:])
                rl16 = rlp16.tile([16, 512], F32, tag="rlf")
                nc.vector.scalar_tensor_tensor(rl16[:], rlo_ps[:], br16[:, 0:1],
                                               mlp16[:], op0=OP.add, op1=OP.add)
                for t4 in range(4):
                    tp = ps_den.tile([128, 16], F32, tag="den")
                    nc.tensor.transpose(tp[:, 0:16],
                                        rl16[:, t4 * 128:(t4 + 1) * 128],
                                        ident[0:16, 0:16])
                    rt = fin.tile([128, 16], F32, tag="fin")
                    nc.vector.tensor_copy(rt[:], tp[:, 0:16])
                    rl8 = rt[:, 0:8]
                    h8 = rt[:, 8:16]
                    m1 = fin.tile([128, 1], F32, tag="fin1")
                    nc.vector.tensor_reduce(m1[:], rl8, AX.X, OP.max)
                    eq1 = fin.tile([128, 8], dt.int32, tag="fini")
                    nc.vector.tensor_scalar(eq1[:], rl8, m1[:], None,
                                            op0=OP.is_equal)
                    cand = fin.tile([128, 8], F32, tag="fin")
                    nc.vector.select(cand[:], eq1[:], iota8b[:], c99[:])
                    idx1 = fin.tile([128, 1], F32, tag="fin1")
                    nc.vector.tensor_reduce(idx1[:], cand[:], AX.X, OP.min)
                    eqi1 = fin.tile([128, 8], dt.int32, tag="fini")
                    nc.vector.tensor_scalar(eqi1[:], iota8b[:], idx1[:], None,
                                            op0=OP.is_equal)
                    sel1 = fin.tile([128, 8], F32, tag="fin")
                    nc.vector.select(sel1[:], eqi1[:], h8, zero8[:])
                    v1 = fin.tile([128, 1], F32, tag="fin1")
                    nc.vector.tensor_reduce(v1[:], sel1[:], AX.X, OP.add)
                    rl8b = fin.tile([128, 8], F32, tag="fin")
                    nc.vector.select(rl8b[:], eqi1[:], negb[:], rl8)
                    m2 = fin.tile([128, 1], F32, tag="fin1")
                    nc.vector.tensor_reduce(m2[:], rl8b[:], AX.X, OP.max)
                    eq2 = fin.tile([128, 8], dt.int32, tag="fini")
                    nc.vector.tensor_scalar(eq2[:], rl8b[:], m2[:], None,
                                            op0=OP.is_equal)
                    cand2 = fin.tile([128, 8], F32, tag="fin")
                    nc.vector.select(cand2[:], eq2[:], iota8b[:], c99[:])
                    idx2 = fin.tile([128, 1], F32, tag="fin1")
                    nc.vector.tensor_reduce(idx2[:], cand2[:], AX.X, OP.min)
                    eqi2 = fin.tile([128, 8], dt.int32, tag="fini")
                    nc.vector.tensor_scalar(eqi2[:], iota8b[:], idx2[:], None,
                                            op0=OP.is_equal)
                    sel2 = fin.tile([128, 8], F32, tag="fin")
                    nc.vector.select(sel2[:], eqi2[:], h8, zero8[:])
                    v2 = fin.tile([128, 1], F32, tag="fin1")
                    nc.vector.tensor_reduce(v2[:], sel2[:], AX.X, OP.add)
                    agg = fin.tile([128, 1], F32, tag="fin1")
                    nc.vector.tensor_tensor(agg[:], v1[:], v2[:], op=OP.add)
                    outt = otp.tile([128, 512], F32, tag="ot")
                    nc.vector.scalar_tensor_tensor(outt[:], wsumb[:], agg[:],
                                                   boutb[:], op0=OP.mult,
                                                   op1=OP.add)
                    nc.gpsimd.dma_start(
                        out_d[c * 512 + t4 * 128: c * 512 + (t4 + 1) * 128, :],
                        outt[:])
    nc.compile()
    return nc


def _prepare_inputs(inputs):
    f32 = np.float32
    inp = {k: np.asarray(v, f32) for k, v in inputs.items()}
    ln1, ln2 = inp["ln1_w"], inp["ln2_w"]
    Wq_f = ln1[:, None] * inp["Wq"]
    Wk_f = ln1[:, None] * inp["Wk"] / np.sqrt(DH)
    Wv_f = ln1[:, None] * inp["Wv"]
    Wg_f = np.zeros((D, FFP), f32); Wg_f[:, :FF] = ln2[:, None] * inp["W_gate"]
    Wu_f = np.zeros((D, FFP), f32); Wu_f[:, :FF] = ln2[:, None] * inp["W_up"]
    Wds = np.zeros((FFP, 16), f32)
    Wds[:FF, :8] = inp["W_down"] @ inp["W_router"]
    Wds[:FF, 8:] = inp["W_down"][:, :8]
    Wrs = np.zeros((D, 16), f32)
    Wrs[:, :8] = inp["W_router"]; Wrs[:8, 8:] = np.eye(8, dtype=f32)
    wsum = 0.5 * inp["W_out"].sum(0)

    xT = inp["x"].reshape(NT, DIN).T.copy()
    masks = np.zeros((4, 128, 512), f32)
    jj = np.arange(128)[:, None]; ii = np.arange(512)[None, :]
    for dx in range(4):
        masks[dx] = (jj + dx * 128 <= ii)

    def c(a):
        return np.ascontiguousarray(a)

    common = {
        "xT16": c(_split(xT.reshape(4, 128, NT))),
        "Win16": c(_split(inp["W_in"].reshape(4, 128, 32, 128)
                          .transpose(2, 1, 0, 3))),
        "masks16": c(masks.transpose(1, 0, 2).astype(BF)),
        "bin_t": c(inp["b_in"].reshape(32, 128).T),
        "br16": np.concatenate([inp["b_router"],
                                np.zeros(8, f32)])[:, None].copy(),
        "wsumb": c(np.tile(wsum[None, :], (128, 1))),
        "boutb": c(np.tile(inp["b_out"][None, :], (128, 1))),
        "iota8b": c(np.tile(np.arange(8, dtype=f32)[None, :], (128, 1))),
        "ident": np.eye(128, dtype=f32),
    }
    in_maps = []
    for r in range(NCORE):
        hs = slice(r * HPC * DH, (r + 1) * HPC * DH)
        fs = slice(r * FFS, (r + 1) * FFS)
        m = dict(common)
        m["Wq16"] = c(_split(Wq_f[:, hs].reshape(32, 128, 4, 128)
                             .transpose(2, 1, 0, 3)))
        m["Wk16"] = c(_split(Wk_f[:, hs].reshape(32, 128, 4, 128)
                             .transpose(2, 1, 0, 3)))
        m["Wv16"] = c(_split(Wv_f[:, hs].reshape(32, 128, 512)))
        m["Wo16"] = c(_split(inp["Wo"][hs, :].reshape(4, 128, 32, 128)
                             .transpose(2, 1, 0, 3)))
        m["Wg16"] = c(_split(Wg_f[:, fs].reshape(32, 128, 11, 128)
                             .transpose(2, 1, 0, 3)))
        m["Wu16"] = c(_split(Wu_f[:, fs].reshape(32, 128, 11, 128)
                             .transpose(2, 1, 0, 3)))
        m["Wds16"] = c(_split(Wds[fs, :].reshape(11, 128, 16)
                              .transpose(1, 0, 2)))
        m["Wrs16"] = c(_split(Wrs.reshape(32, 128, 16).transpose(1, 0, 2)))
        in_maps.append(m)
    return in_maps


_NC_CACHE = None


def kernel(**inputs):
    global LAST_RESULT, _NC_CACHE
    if _NC_CACHE is None:
        _NC_CACHE = _build()
    in_maps = _prepare_inputs(inputs)
    res = run_bass_kernel_spmd(_NC_CACHE, in_maps, core_ids=list(range(NCORE)))
    LAST_RESULT = res
    return res.results[0]["out"].reshape(B, S, 512).astype(np.float32)
